# revision 68
# baseline (speedup 1.0000x reference)
"""Block-causal attention Trainium2 kernel (8 NeuronCores), v3.

Sharding: core c = b*4 + g handles batch b (of 2) and head-group g (4 of 16
heads). Each core computes the qkv projection, rmsnorm + 2-D RoPE,
block-causal attention and a partial output projection for its 256 channels;
the host sums the 4 per-group partials per batch (bf16 partials).

v3 (222.6us -> ~196us). NOTE: the walrus schedule is sensitive to source
  LINE NUMBERS -- edits below reshuffle it +-5us; keep line counts fixed.
  merged exps: per-kt head-pair scores go into one [128,2,512] psum tile;
    ONE exp covers both heads (72x1067ns=77us, the serial wall; tail-pair
    kts use the same shape).
  normalize via ONE gpsimd partition_broadcast (no skbd DRAM round-trip).
  psum: stp 2x2-bank + pvp 2 + pps 2 = 8 banks; head-pairs driven
    sequentially (hp0 fully, then hp1) so 2 pv banks suffice.
  pull-forward: att(fp) starts during lc=fp (old keys, capped at 4*fp kts
    until KPl[lc] folds); fin(fp) enters the driver a few steps into fp+1.
  pre-tiled DRAM (contiguous [128,512] tiles for x/wqk/wv/wo/out) kills the
    startup descriptor storm; 60 warmup matmuls (stationary from a gpsimd
    memset -> start at engine-go) keep the PE HAM-warm through DMA wait.
  rms skbq round-trip on sync DMA rings (parallel ~2us, not ~5us/lc on
    Pool); x prefetched 2 deep in split halves; V-copies fused [128,4x64].
A/B facts: fin interleaved INTO att2 (via pps ring) head-blocks the tensor
  queue; K-fold must trail V-proj or its Rk_b wait head-blocks the vector
  queue; <60 warmups lets the HAM re-throttle early (+8us).
Hardware pitfalls baked in: tile_position column-tiling produces garbage on
  this HW; multi-bank PSUM *matmul out* tiles are fine per-bank-slice and a
  single ACT may READ across 2 banks (verified); partition_broadcast only
  sources partition 0; reciprocal_approx_fast needs multi-row base-0 fp32
  APs; GPSIMD (Pool) cannot touch PSUM, only copy/memset-class ops.
"""

import os
import numpy as np

import concourse.bass as bass
import concourse.mybir as mybir
import concourse.tile as tile
from concourse import bacc
from concourse.bass_utils import run_bass_kernel_spmd

F32 = mybir.dt.float32
BF16 = mybir.dt.bfloat16
AF = mybir.ActivationFunctionType
ALU = mybir.AluOpType
MUL = ALU.mult
ADD = ALU.add
SUB = ALU.subtract

B, T, NP, D, H = 2, 8, 256, 1024, 16
L = T * NP            # 2048
HD = 64               # head dim
HPG = 4               # heads per group (4 groups x 2 batches = 8 cores)
CPG = HPG * HD        # 256 channels per group
NDT = D // 128        # 8 d-tiles
NLC = L // 512        # 4 l-chunks (= frame pairs)
NLT = L // 128        # 16 l-tiles
EPS = 1e-6

_CACHE = {}


class Driver:
    """FIFO generator driver with per-entry caps and delayed insertion."""

    def __init__(self):
        self.q = []        # [gen, cap_or_None]
        self.delayed = []  # [steps_left, gen]

    def add(self, gen, cap=None):
        self.q.append([gen, cap])

    def add_delayed(self, gen, after=4):
        self.delayed.append([after, gen])

    def uncap_all(self):
        for e in self.q:
            e[1] = None

    def step(self, n=1):
        done = 0
        while done < n:
            ent = None
            for e in self.q:
                if e[1] is None or e[1] > 0:
                    ent = e
                    break
            if ent is None:
                if self.delayed:
                    d = self.delayed.pop(0)
                    self.q.append([d[1], None])
                    continue
                return done
            r = next(ent[0], "END")
            if r == "END":
                self.q.remove(ent)
            elif ent[1] is not None:
                ent[1] -= 1
            done += 1
            for d in self.delayed:
                d[0] -= 1
            while self.delayed and self.delayed[0][0] <= 0:
                d = self.delayed.pop(0)
                self.q.append([d[1], None])
        return done

    def drain(self):
        while self.step(32):
            pass


def _emit(nc, tc, ctx, xT0, xTr, wqk, wv, wo, wvec, costab, sintab, out, skbq):
    sing = ctx.enter_context(tc.tile_pool(name="sing", bufs=1))
    xp = ctx.enter_context(tc.tile_pool(name="xp", bufs=3))
    tmp = ctx.enter_context(tc.tile_pool(name="tmp", bufs=4))
    sqp = ctx.enter_context(tc.tile_pool(name="sqp", bufs=6))
    ptp = ctx.enter_context(tc.tile_pool(name="ptp", bufs=6))
    osb = ctx.enter_context(tc.tile_pool(name="osb", bufs=4))
    rdp = ctx.enter_context(tc.tile_pool(name="rdp", bufs=2))
    # PSUM: pps 2 + stp 2x2 + pvp 2 = 8 banks
    pps = ctx.enter_context(tc.tile_pool(name="pps", bufs=2, space="PSUM"))
    stp = ctx.enter_context(tc.tile_pool(name="stp", bufs=2, space="PSUM"))
    pvp = ctx.enter_context(tc.tile_pool(name="pvp", bufs=2, space="PSUM"))

    # ---- persistent SBUF; first-needed first ----
    wvec_sb = sing.tile([128, 32], BF16)
    nc.sync.dma_start(out=wvec_sb[:], in_=wvec[:])
    # warmup matmuls: keep the PE HAM-warm through the input DMA wait; the
    # stationary comes from a gpsimd memset (that queue is empty at start)
    wrm = sing.tile([128, 128], BF16)
    nc.gpsimd.memset(wrm[:], 0.125)
    wps = pps.tile([128, 128], F32, name="wps", tag="ps")
    for i in range(60):
        nc.tensor.matmul(wps[:], wrm[:], wrm[:], start=True, stop=True,
                         skip_group_check=True)
    # x0/wqk split into 64KB pieces so they spread over all DMA rings and
    # land first; everything else (x1, wv, cos/sin/wo) is emitted later from
    # the lc0 section body so it can't steal their bandwidth
    wqk_sb = sing.tile([128, NDT, 512], BF16)
    x0 = xp.tile([128, NDT, 512], BF16, name="x0", tag="xt")
    for dt in range(NDT):
        for h in range(2):
            cs = slice(256 * h, 256 * h + 256)
            nc.sync.dma_start(out=x0[:, dt, cs], in_=xT0[dt, :, cs])
            nc.sync.dma_start(out=wqk_sb[:, dt, cs], in_=wqk[dt, :, cs])
    wv_sb = sing.tile([128, NDT, CPG], BF16)
    # cos/sin/wo are loaded from inside the lc0 section (deferred so they
    # don't compete with the startup-critical x0/wqk loads)
    cos_sb = sing.tile([128, L], BF16)
    sin_sb = sing.tile([128, L], BF16)
    wo_sb = sing.tile([128, 2, D], BF16)

    qk_sb = [sing.tile([128, L], BF16, name=f"qk{i}") for i in range(4)]
    rope_sb = [sing.tile([128, L], BF16, name=f"rope{i}") for i in range(4)]
    QPl = [[sing.tile([128, 512], BF16, name=f"qp{i}_{c}") for c in range(NLC)]
           for i in range(2)]
    KPl = [[sing.tile([128, 512], BF16, name=f"kp{i}_{c}") for c in range(NLC)]
           for i in range(2)]
    v_sb = sing.tile([128, NLT, 4, 65], BF16)
    nc.vector.memset(v_sb[:, :, :, 64:65], 1.0)

    epsb = sing.tile([8, 1], F32)
    nc.vector.memset(epsb[:], EPS)

    drv = Driver()

    # ---------------- attention pass (generator, one head-pair) ------------
    def att2(hp, fp):
        nkt_sh, nkt_all = 4 * fp + 2, 4 * fp + 4
        last = nkt_all - 1
        fps = slice(512 * fp, 512 * fp + 512)
        pv = [pvp.tile([65, 512], F32, name=f"pv{hp}_{fp}_{i}", tag="pv")
              for i in range(2)]
        pend = []

        def flush_pv():
            kt_, rhs = pend.pop(0)
            qof_ = 0 if kt_ < nkt_sh else 256
            for i in range(2):
                nc.tensor.matmul(pv[i][:, qof_:512],
                                 v_sb[:, kt_, 2 * hp + i, :], rhs(i),
                                 start=(kt_ == 0), stop=(kt_ == last),
                                 skip_group_check=True)

        for kt in range(nkt_sh):
            lck, kof = kt // 4, (kt % 4) * 128
            st = stp.tile([128, 2, 512], F32, name="st", tag="st")
            for i in range(2):
                nc.tensor.matmul(st[:, i, :],
                                 KPl[hp][lck][64 * i:64 * i + 64, kof:kof + 128],
                                 QPl[hp][fp][64 * i:64 * i + 64, :],
                                 start=True, stop=True, skip_group_check=True)
            pt = ptp.tile([128, 2, 512], BF16, name="pt", tag="pt")
            nc.scalar.activation(pt[:], st[:], AF.Exp)
            pend.append((kt, lambda i, p=pt: p[:, i, 0:512]))
            if len(pend) > 1:
                flush_pv()
            yield
        # trailing two key-tiles (frame f1 only, queries 256:512): both heads
        # and both kts in one [128,2,2,256] tile -> a single exp
        st2 = stp.tile([128, 2, 2, 256], F32, name="st2", tag="st")
        for j in range(2):
            kt = nkt_sh + j
            lck, kof = kt // 4, (kt % 4) * 128
            for i in range(2):
                nc.tensor.matmul(st2[:, i, j, :],
                                 KPl[hp][lck][64 * i:64 * i + 64, kof:kof + 128],
                                 QPl[hp][fp][64 * i:64 * i + 64, 256:512],
                                 start=True, stop=True, skip_group_check=True)
        pt2 = ptp.tile([128, 2, 2, 256], BF16, name="pt2", tag="pt")
        nc.scalar.activation(pt2[:], st2[:], AF.Exp)
        for j in range(2):
            pend.append((nkt_sh + j, lambda i, j=j: pt2[:, i, j, :]))
        yield
        while pend:
            flush_pv()
            yield
        # denominators -> reciprocal -> partition broadcast -> normalize
        dstg = tmp.tile([33, 1024], F32, tag="dc")
        nc.gpsimd.memset(dstg[:], 1.0)
        for i in range(2):
            nc.vector.tensor_scalar(out=dstg[0:1, 512 * i:512 * i + 512],
                                    in0=pv[i][64:65, :],
                                    scalar1=1e-30, scalar2=None, op0=ALU.max)
        dr = tmp.tile([33, 1024], F32, tag="dc")
        nc.vector.reciprocal_approx_fast(out=dr[:], in_=dstg[:])
        rd = rdp.tile([64, 1024], F32, tag="rd")
        nc.gpsimd.partition_broadcast(rd[:], dr[0:1, :])
        for i in range(2):
            nc.vector.scalar_tensor_tensor(out=att_sb[hp][64 * i:64 * i + 64, fps],
                                           in0=pv[i][0:64, :], scalar=1.0,
                                           in1=rd[:, 512 * i:512 * i + 512],
                                           op0=MUL, op1=MUL)
        yield

    att_sb = [sing.tile([128, L], BF16, name=f"att{i}") for i in range(2)]

    def finish_fp(fp):
        # casts go to the ACT engine when it has queue slack (copy lives in
        # the same table set as exp, so no table switch)
        on_scalar = True
        for lt4 in range(4):
            lt = fp * 4 + lt4
            lsl = slice(lt * 128, lt * 128 + 128)
            for oc in range(2):
                ocs = slice(oc * 512, oc * 512 + 512)
                ps = pps.tile([128, 512], F32, name="ops", tag="ps")
                for ct in range(2):
                    nc.tensor.matmul(ps[:], att_sb[ct][:, lsl],
                                     wo_sb[:, ct, ocs], start=(ct == 0),
                                     stop=(ct == 1))
                ob = osb.tile([128, 512], BF16, tag="ob")
                if on_scalar:
                    nc.scalar.copy(ob[:], ps[:])
                else:
                    nc.vector.tensor_copy(ob[:], ps[:])
                nc.sync.dma_start(out=out[lt, oc], in_=ob[:])
            yield

    # ---------------- main loop over l-chunks ------------------------------
    def prefetch(nlc):
        xn = xp.tile([128, NDT, 512], BF16, name=f"x{nlc}", tag="xt")
        for dt2 in range(0, NDT, 2):
            nc.sync.dma_start(out=xn[:, dt2:dt2 + 2, :],
                              in_=xTr[nlc - 1, :, dt2:dt2 + 2, :])
        return xn

    xtiles = {0: x0}
    for lc in range(NLC):
        ls = slice(lc * 512, (lc + 1) * 512)
        xt = xtiles.pop(lc)

        if lc == 0:
            for q4 in range(4):
                qs = slice(512 * q4, 512 * q4 + 512)
                nc.scalar.dma_start(out=cos_sb[:, qs], in_=costab[:, qs])
                nc.scalar.dma_start(out=sin_sb[:, qs], in_=sintab[:, qs])
            nc.scalar.dma_start(out=wo_sb[:, 0, :], in_=wo[:, 0, :])
            nc.scalar.dma_start(out=wo_sb[:, 1, :], in_=wo[:, 1, :])
            nc.sync.dma_start(out=wv_sb[:, 0:4, :], in_=wv[:, 0:4, :])
            nc.sync.dma_start(out=wv_sb[:, 4:8, :], in_=wv[:, 4:8, :])

        sqs = []
        for pair in range(2):                    # 0: q (ot 0,1), 1: k (ot 2,3)
            for comp in range(2):
                drv.step(3)
                ot = pair * 2 + comp
                ps = pps.tile([128, 512], F32, name="qkps", tag="ps")
                for dt in range(NDT):
                    nc.tensor.matmul(ps[:], wqk_sb[:, dt, ot * 128:(ot + 1) * 128],
                                     xt[:, dt, :], start=(dt == 0),
                                     stop=(dt == NDT - 1))
                nc.vector.tensor_copy(qk_sb[ot][:, ls], ps[:])
                sq = sqp.tile([128, 512], BF16, name="sq", tag="sq")
                nc.vector.tensor_tensor(sq[:], qk_sb[ot][:, ls],
                                        qk_sb[ot][:, ls], MUL)
                sqs.append(sq)
        if lc + 1 < NLC and lc + 1 not in xtiles:
            xtiles[lc + 1] = prefetch(lc + 1)
        drv.step(2)

        # rms sums: one [8,512] bank; q heads rows 0:4, k heads rows 4:8
        rsum = pps.tile([8, 512], F32, name="rsum", tag="ps")
        for c in range(4):
            nc.tensor.matmul(rsum[:, :], wvec_sb[:, 8 * c:8 * c + 8],
                             sqs[c][:], start=(c == 0), stop=(c == 3),
                             skip_group_check=True)
        # the rms -> round-trip -> rope -> fold chain gates the next fp's exp
        # stream; high_priority stops the list scheduler from parking it
        # behind bulk casts (observed +15us on the first fold)
        with tc.high_priority():
            rln = tmp.tile([8, 512], F32, tag="rln")
            nc.scalar.activation(rln[:], rsum[:], AF.Ln, bias=epsb[:])
            rqs = tmp.tile([8, 512], BF16, tag="rqs")
            nc.scalar.activation(rqs[:], rln[:], AF.Exp, scale=-0.5)
            nc.sync.dma_start(out=skbq[0:8, ls], in_=rqs[0:8, :])
            Rq_b = sing.tile([128, 512], BF16, name=f"rqb{lc}")
            for h in range(4):
                nc.sync.dma_start(out=Rq_b[32 * h:32 * h + 32, :],
                                  in_=skbq[h:h + 1, ls].to_broadcast((32, 512)))
            Rk_b = []
            for hp in range(2):
                rkb = sing.tile([128, 512], BF16, name=f"rkb{lc}_{hp}")
                for i in range(2):
                    nc.sync.dma_start(
                        out=rkb[64 * i:64 * i + 64, :],
                        in_=skbq[4 + 2 * hp + i:5 + 2 * hp + i, ls]
                        .to_broadcast((64, 512)))
                Rk_b.append(rkb)

        # per-lc RoPE, all bf16. K side first and folded immediately (it has
        # no rms-round-trip dependency beyond Rk_b), then V-proj fills the
        # time the Rq_b broadcast needs to land, then the Q side; att2(lc) is
        # born with every dependency already emitted (no caps needed).
        def rope_side(base):
            xr, xi = qk_sb[base][:, ls], qk_sb[base + 1][:, ls]
            for comp in range(2):
                drv.step(2)
                with tc.high_priority():
                    t1 = tmp.tile([128, 512], BF16, tag="t1")
                    t2 = tmp.tile([128, 512], BF16, tag="t2")
                    ca, cb = (cos_sb, sin_sb) if comp == 0 else (sin_sb, cos_sb)
                    nc.vector.tensor_tensor(t1[:], xr, ca[:, ls], MUL)
                    nc.vector.tensor_tensor(t2[:], xi, cb[:, ls], MUL)
                    op = SUB if comp == 0 else ADD
                    dst = rope_sb[base + comp][:, ls]
                    if base == 0:
                        t3 = tmp.tile([128, 512], BF16, tag="t3")
                        nc.vector.tensor_tensor(t3[:], t1[:], t2[:], op)
                        nc.vector.tensor_tensor(dst, t3[:], Rq_b[:], MUL)
                    else:
                        nc.vector.tensor_tensor(dst, t1[:], t2[:], op)
            srcs = (rope_sb[0], rope_sb[1]) if base == 0 else \
                   (rope_sb[2], rope_sb[3])
            dstt = QPl if base == 0 else KPl
            with tc.high_priority():
                for hp2 in range(2):
                    for i2 in range(2):
                        h2 = hp2 * 2 + i2
                        nc.sync.dma_start(
                            out=dstt[hp2][lc][64 * i2:64 * i2 + 32, :],
                            in_=srcs[0][32 * h2:32 * h2 + 32, ls])
                        nc.sync.dma_start(
                            out=dstt[hp2][lc][64 * i2 + 32:64 * i2 + 64, :],
                            in_=srcs[1][32 * h2:32 * h2 + 32, ls])

        # V projection: l on partitions; 2 chains share one bank
        for vt in range(2):
            drv.step(3)
            vps = pps.tile([128, 2, CPG], F32, name="vps", tag="ps")
            for c2 in range(2):
                ls4 = vt * 2 + c2
                for dt in range(NDT):
                    nc.tensor.matmul(vps[:, c2, :],
                                     xt[:, dt, ls4 * 128:(ls4 + 1) * 128],
                                     wv_sb[:, dt, :],
                                     start=(c2 == 0 and dt == 0),
                                     stop=(c2 == 1 and dt == NDT - 1),
                                     skip_group_check=True)
            for c2 in range(2):
                lt = lc * 4 + vt * 2 + c2
                nc.vector.tensor_copy(v_sb[:, lt, :, 0:64], vps[:, c2, :])
        if lc + 2 < NLC:
            xtiles[lc + 2] = prefetch(lc + 2)

        rope_side(2)
        rope_side(0)
        # fp=lc attention can start on old-key tiles now (capped until fold)
        drv.add(att2(0, lc), cap=4 * lc)
        drv.add(att2(1, lc), cap=4 * lc)
        drv.step(2)
        # fold 0.125*r_k into K (per head rows), in place
        with tc.high_priority():
            for hp2 in range(2):
                nc.vector.scalar_tensor_tensor(
                    out=KPl[hp2][lc][:], in0=KPl[hp2][lc][:], scalar=0.125,
                    in1=Rk_b[hp2][:], op0=MUL, op1=MUL)
        drv.uncap_all()
        # fin(fp=lc-1) a few steps into fp=lc's stream so its first matmuls
        # never head-block the tensor queue on the normalize chain
        if lc >= 1:
            drv.add_delayed(finish_fp(lc - 1), after=6)
        drv.step(4)

    # tail: drain everything, then the last out-projection
    drv.add_delayed(finish_fp(3), after=6)
    drv.drain()


def _build_nc():
    import contextlib
    from concourse import hw_specs as _hw

    # Route every Exp/Ln activation to the one table set that holds both
    # (natural_log_exp_and_others), so the per-lc rms Ln doesn't thrash the
    # ACT tables against the attention Exp. Only set *contents* are filtered;
    # list order/length is preserved so act_func_set_id indices stay valid.
    _orig_gat = _hw.get_activation_tables

    def _gat(arch):
        t = _orig_gat(arch)
        both = mybir.ActivationFunctionType.Exp, mybir.ActivationFunctionType.Ln
        return {name: (fns if name == "natural_log_exp_and_others"
                       else fns - set(both))
                for name, fns in t.items()}

    bacc.get_activation_tables = _gat
    nc = bacc.Bacc("TRN2", target_bir_lowering=False, debug=False, num_devices=8)
    xT0 = nc.dram_tensor("xT0", (NDT, 128, 512), BF16, kind="ExternalInput")
    xTr = nc.dram_tensor("xTr", (NLC - 1, 128, NDT, 512), BF16,
                         kind="ExternalInput")
    wqk = nc.dram_tensor("wqk", (NDT, 128, 512), BF16, kind="ExternalInput")
    wv = nc.dram_tensor("wv", (128, NDT, CPG), BF16, kind="ExternalInput")
    wo = nc.dram_tensor("wo", (128, 2, D), BF16, kind="ExternalInput")
    wvec = nc.dram_tensor("wvec", (128, 32), BF16, kind="ExternalInput")
    costab = nc.dram_tensor("costab", (128, L), BF16, kind="ExternalInput")
    sintab = nc.dram_tensor("sintab", (128, L), BF16, kind="ExternalInput")
    out = nc.dram_tensor("out", (NLT, 2, 128, 512), BF16, kind="ExternalOutput")
    skbq = nc.dram_tensor("skbq", (8, L), BF16)

    with tile.TileContext(nc) as tc, contextlib.ExitStack() as ctx:
        _emit(nc, tc, ctx, xT0.ap(), xTr.ap(), wqk.ap(), wv.ap(), wo.ap(),
              wvec.ap(), costab.ap(), sintab.ap(), out.ap(), skbq.ap())
    nc.compile()
    bacc.get_activation_tables = _orig_gat
    return nc


def _host_prep(x, Wqkv, Wout, q_scale, k_scale):
    x = np.asarray(x, np.float32)
    Wqkv = np.asarray(Wqkv, np.float32)
    Wout = np.asarray(Wout, np.float32)
    q_scale = np.asarray(q_scale, np.float32)
    k_scale = np.asarray(k_scale, np.float32)

    quarter = HD // 4  # 16
    inv = 1.0 / (10000.0 ** (np.arange(quarter, dtype=np.float64) / quarter))
    tt = np.repeat(np.arange(T), NP).astype(np.float64)
    pp = np.tile(np.arange(NP), T).astype(np.float64)
    ang = np.concatenate([tt[:, None] * inv[None, :], pp[:, None] * inv[None, :]],
                         axis=1)  # (L, 32)
    costab = np.tile(np.cos(ang).astype(np.float32).T, (4, 1))  # (128, L)
    sintab = np.tile(np.sin(ang).astype(np.float32).T, (4, 1))

    import ml_dtypes
    ev, od = np.arange(0, HD, 2), np.arange(1, HD, 2)
    # four [128,8] rms stationaries (qR,qI,kR,kI); q heads cols 0:4 of each
    # block, k heads cols 4:8; zero-padded so all mms share out rows 0:8
    wvec = np.zeros((128, 32), np.float32)
    for hh in range(HPG):
        r = slice(32 * hh, 32 * hh + 32)
        wvec[r, 0 + hh] = 1.0 / (HD * q_scale[ev] ** 2)
        wvec[r, 8 + hh] = 1.0 / (HD * q_scale[od] ** 2)
        wvec[r, 16 + 4 + hh] = 1.0 / (HD * k_scale[ev] ** 2)
        wvec[r, 24 + 4 + hh] = 1.0 / (HD * k_scale[od] ** 2)

    in_maps = []
    for c in range(8):
        b, g = c // 4, c % 4
        wqk = np.empty((D, 512), np.float32)
        for hh in range(HPG):
            gh = g * HPG + hh
            wq = Wqkv[gh * HD:(gh + 1) * HD, :] * q_scale[:, None]
            wk = Wqkv[D + gh * HD:D + (gh + 1) * HD, :] * k_scale[:, None]
            wqk[:, 0 + 32 * hh:32 + 32 * hh] = wq[ev].T
            wqk[:, 128 + 32 * hh:160 + 32 * hh] = wq[od].T
            wqk[:, 256 + 32 * hh:288 + 32 * hh] = wk[ev].T
            wqk[:, 384 + 32 * hh:416 + 32 * hh] = wk[od].T
        # pre-tiled layouts: contiguous per-tile DRAM so the DMA descriptor
        # stream stays compact
        wqk_t = np.ascontiguousarray(wqk.reshape(NDT, 128, 512))
        wv_m = Wqkv[2 * D + g * CPG:2 * D + (g + 1) * CPG, :].T  # (D, CPG)
        wv_t = np.ascontiguousarray(wv_m.reshape(NDT, 128, CPG).transpose(1, 0, 2))
        wo_m = Wout[:, g * CPG:(g + 1) * CPG].T                  # (CPG, D)
        wo_t = np.ascontiguousarray(wo_m.reshape(2, 128, D).transpose(1, 0, 2))
        xT = x[b].T                                              # (D, L)
        # lc0 dt-major contiguous tiles; lc1-3 p-major (one dma per chunk)
        x0_t = np.ascontiguousarray(
            xT[:, 0:512].reshape(NDT, 128, 512))
        xr_t = np.ascontiguousarray(
            xT[:, 512:].reshape(NDT, 128, NLC - 1, 512).transpose(2, 1, 0, 3))
        in_maps.append({
            "xT0": x0_t.astype(ml_dtypes.bfloat16),
            "xTr": xr_t.astype(ml_dtypes.bfloat16),
            "wqk": wqk_t.astype(ml_dtypes.bfloat16),
            "wv": wv_t.astype(ml_dtypes.bfloat16),
            "wo": wo_t.astype(ml_dtypes.bfloat16),
            "wvec": wvec.astype(ml_dtypes.bfloat16),
            "costab": costab.astype(ml_dtypes.bfloat16),
            "sintab": sintab.astype(ml_dtypes.bfloat16),
        })
    return in_maps


def kernel(x, Wqkv, Wout, q_scale, k_scale, T=None, N_p=None):
    assert int(T) == 8 and int(N_p) == 256
    if "nc" not in _CACHE:
        _CACHE["nc"] = _build_nc()
    nc = _CACHE["nc"]
    in_maps = _host_prep(x, Wqkv, Wout, q_scale, k_scale)
    trace = bool(int(os.environ.get("KERNEL_TRACE", "0")))
    res = run_bass_kernel_spmd(nc, in_maps, core_ids=list(range(8)), trace=trace)
    _CACHE["last_exec_time_ns"] = res.exec_time_ns
    outp = np.zeros((B, L, D), np.float32)
    for c in range(8):
        o = np.asarray(res.results[c]["out"], np.float32)  # (NLT, 2, 128, 512)
        outp[c // 4] += o.transpose(0, 2, 1, 3).reshape(L, D)
    return outp


if __name__ == "__main__":
    rng = np.random.default_rng(0)
    x = rng.standard_normal((B, L, D), dtype=np.float32)
    Wqkv = rng.standard_normal((3 * D, D), dtype=np.float32) * 0.02
    Wout = rng.standard_normal((D, D), dtype=np.float32) * 0.02
    o = kernel(x, Wqkv, Wout, np.ones(HD, np.float32), np.ones(HD, np.float32),
               8, 256)
    print("out", o.shape, o.dtype, float(np.abs(o).mean()))


# revision 69
# speedup vs baseline: 1.1981x; 1.1981x over previous
"""Block-causal attention Trainium2 kernel (8 NeuronCores), v3.

Sharding: core c = b*4 + g handles batch b (of 2) and head-group g (4 of 16
heads). Each core computes the qkv projection, rmsnorm + 2-D RoPE,
block-causal attention and a partial output projection for its 256 channels;
the host sums the 4 per-group partials per batch (bf16 partials).

v3 (222.6us -> ~196us). NOTE: the walrus schedule is sensitive to source
  LINE NUMBERS -- edits below reshuffle it +-5us; keep line counts fixed.
  merged exps: per-kt head-pair scores go into one [128,2,512] psum tile;
    ONE exp covers both heads (72x1067ns=77us, the serial wall; tail-pair
    kts use the same shape).
  normalize via ONE gpsimd partition_broadcast (no skbd DRAM round-trip).
  psum: stp 2x2-bank + pvp 2 + pps 2 = 8 banks; head-pairs driven
    sequentially (hp0 fully, then hp1) so 2 pv banks suffice.
  pull-forward: att(fp) starts during lc=fp (old keys, capped at 4*fp kts
    until KPl[lc] folds); fin(fp) enters the driver a few steps into fp+1.
  pre-tiled DRAM (contiguous [128,512] tiles for x/wqk/wv/wo/out) kills the
    startup descriptor storm; 60 warmup matmuls (stationary from a gpsimd
    memset -> start at engine-go) keep the PE HAM-warm through DMA wait.
  rms skbq round-trip on sync DMA rings (parallel ~2us, not ~5us/lc on
    Pool); x prefetched 2 deep in split halves; V-copies fused [128,4x64].
A/B facts: fin interleaved INTO att2 (via pps ring) head-blocks the tensor
  queue; K-fold must trail V-proj or its Rk_b wait head-blocks the vector
  queue; <60 warmups lets the HAM re-throttle early (+8us).
Hardware pitfalls baked in: tile_position column-tiling produces garbage on
  this HW; multi-bank PSUM *matmul out* tiles are fine per-bank-slice and a
  single ACT may READ across 2 banks (verified); partition_broadcast only
  sources partition 0; reciprocal_approx_fast needs multi-row base-0 fp32
  APs; GPSIMD (Pool) cannot touch PSUM, only copy/memset-class ops.
"""

import os
import numpy as np

import concourse.bass as bass
import concourse.mybir as mybir
import concourse.tile as tile
from concourse import bacc
from concourse.bass_utils import run_bass_kernel_spmd

F32 = mybir.dt.float32
BF16 = mybir.dt.bfloat16
AF = mybir.ActivationFunctionType
ALU = mybir.AluOpType
MUL = ALU.mult
ADD = ALU.add
SUB = ALU.subtract

B, T, NP, D, H = 2, 8, 256, 1024, 16
L = T * NP            # 2048
HD = 64               # head dim
HPG = 4               # heads per group (4 groups x 2 batches = 8 cores)
CPG = HPG * HD        # 256 channels per group
NDT = D // 128        # 8 d-tiles
NLC = L // 512        # 4 l-chunks (= frame pairs)
NLT = L // 128        # 16 l-tiles
EPS = 1e-6

_CACHE = {}


class Driver:
    """FIFO generator driver with per-entry caps and delayed insertion."""

    def __init__(self):
        self.q = []        # [gen, cap_or_None]
        self.delayed = []  # [steps_left, gen]

    def add(self, gen, cap=None):
        self.q.append([gen, cap])

    def add_delayed(self, gen, after=4):
        self.delayed.append([after, gen])

    def uncap_all(self):
        for e in self.q:
            e[1] = None

    def step(self, n=1):
        done = 0
        while done < n:
            ent = None
            for e in self.q:
                if e[1] is None or e[1] > 0:
                    ent = e
                    break
            if ent is None:
                if self.delayed:
                    d = self.delayed.pop(0)
                    self.q.append([d[1], None])
                    continue
                return done
            r = next(ent[0], "END")
            if r == "END":
                self.q.remove(ent)
            elif ent[1] is not None:
                ent[1] -= 1
            done += 1
            for d in self.delayed:
                d[0] -= 1
            while self.delayed and self.delayed[0][0] <= 0:
                d = self.delayed.pop(0)
                self.q.append([d[1], None])
        return done

    def drain(self):
        while self.step(32):
            pass


def _emit(nc, tc, ctx, xT0, xTr, wqk, wv, wo, wvec, costab, sintab, out, skbq):
    sing = ctx.enter_context(tc.tile_pool(name="sing", bufs=1))
    xp = ctx.enter_context(tc.tile_pool(name="xp", bufs=3))
    tmp = ctx.enter_context(tc.tile_pool(name="tmp", bufs=4))
    sqp = ctx.enter_context(tc.tile_pool(name="sqp", bufs=6))
    ptp = ctx.enter_context(tc.tile_pool(name="ptp", bufs=6))
    osb = ctx.enter_context(tc.tile_pool(name="osb", bufs=4))
    rdp = ctx.enter_context(tc.tile_pool(name="rdp", bufs=2))
    # PSUM: pps 2 + stp 2x2 + pvp 2 = 8 banks
    pps = ctx.enter_context(tc.tile_pool(name="pps", bufs=2, space="PSUM"))
    stp = ctx.enter_context(tc.tile_pool(name="stp", bufs=2, space="PSUM"))
    pvp = ctx.enter_context(tc.tile_pool(name="pvp", bufs=2, space="PSUM"))

    # ---- persistent SBUF; first-needed first ----
    wvec_sb = sing.tile([128, 32], BF16)
    nc.sync.dma_start(out=wvec_sb[:], in_=wvec[:])
    # warmup matmuls: keep the PE HAM-warm through the input DMA wait; the
    # stationary comes from a gpsimd memset (that queue is empty at start)
    wrm = sing.tile([128, 128], BF16)
    nc.gpsimd.memset(wrm[:], 0.125)
    wps = pps.tile([128, 128], F32, name="wps", tag="ps")
    for i in range(60):
        nc.tensor.matmul(wps[:], wrm[:], wrm[:], start=True, stop=True,
                         skip_group_check=True)
    # x0/wqk split into 64KB pieces so they spread over all DMA rings and
    # land first; everything else (x1, wv, cos/sin/wo) is emitted later from
    # the lc0 section body so it can't steal their bandwidth
    wqk_sb = sing.tile([128, NDT, 512], BF16)
    x0 = xp.tile([128, NDT, 512], BF16, name="x0", tag="xt")
    for dt in range(NDT):
        for h in range(2):
            cs = slice(256 * h, 256 * h + 256)
            nc.sync.dma_start(out=x0[:, dt, cs], in_=xT0[dt, :, cs])
            nc.sync.dma_start(out=wqk_sb[:, dt, cs], in_=wqk[dt, :, cs])
    wv_sb = sing.tile([128, NDT, CPG], BF16)
    # cos/sin/wo are loaded from inside the lc0 section (deferred so they
    # don't compete with the startup-critical x0/wqk loads)
    cos_sb = sing.tile([128, L], BF16)
    sin_sb = sing.tile([128, L], BF16)
    wo_sb = sing.tile([128, 2, D], BF16)

    qk_sb = [sing.tile([128, L], BF16, name=f"qk{i}") for i in range(4)]
    rope_sb = [sing.tile([128, L], BF16, name=f"rope{i}") for i in range(4)]
    QPl = [[sing.tile([128, 512], BF16, name=f"qp{i}_{c}") for c in range(NLC)]
           for i in range(2)]
    KPl = [[sing.tile([128, 512], BF16, name=f"kp{i}_{c}") for c in range(NLC)]
           for i in range(2)]
    v_sb = sing.tile([128, NLT, 4, 65], BF16)
    nc.vector.memset(v_sb[:, :, :, 64:65], 1.0)

    epsb = sing.tile([8, 1], F32)
    nc.vector.memset(epsb[:], EPS)

    drv = Driver()

    # ---------------- attention pass (generator, one head-pair) ------------
    def att2(hp, fp):
        nkt_sh, nkt_all = 4 * fp + 2, 4 * fp + 4
        last = nkt_all - 1
        fps = slice(512 * fp, 512 * fp + 512)
        pv = [pvp.tile([65, 512], F32, name=f"pv{hp}_{fp}_{i}", tag="pv")
              for i in range(2)]
        pend = []

        def flush_pv():
            kt_, rhs = pend.pop(0)
            qof_ = 0 if kt_ < nkt_sh else 256
            for i in range(2):
                nc.tensor.matmul(pv[i][:, qof_:512],
                                 v_sb[:, kt_, 2 * hp + i, :], rhs(i),
                                 start=(kt_ == 0), stop=(kt_ == last),
                                 skip_group_check=True)

        for kt in range(nkt_sh):
            lck, kof = kt // 4, (kt % 4) * 128
            st = stp.tile([128, 2, 512], F32, name="st", tag="st")
            for i in range(2):
                nc.tensor.matmul(st[:, i, :],
                                 KPl[hp][lck][64 * i:64 * i + 64, kof:kof + 128],
                                 QPl[hp][fp][64 * i:64 * i + 64, :],
                                 start=True, stop=True, skip_group_check=True)
            pt = ptp.tile([128, 2, 512], BF16, name="pt", tag="pt")
            nc.scalar.activation(pt[:], st[:], AF.Exp)
            pend.append((kt, lambda i, p=pt: p[:, i, 0:512]))
            if len(pend) > 1:
                flush_pv()
            yield
        # trailing two key-tiles (frame f1 only, queries 256:512): both heads
        # and both kts in one [128,2,2,256] tile -> a single exp
        st2 = stp.tile([128, 2, 2, 256], F32, name="st2", tag="st")
        for j in range(2):
            kt = nkt_sh + j
            lck, kof = kt // 4, (kt % 4) * 128
            for i in range(2):
                nc.tensor.matmul(st2[:, i, j, :],
                                 KPl[hp][lck][64 * i:64 * i + 64, kof:kof + 128],
                                 QPl[hp][fp][64 * i:64 * i + 64, 256:512],
                                 start=True, stop=True, skip_group_check=True)
        pt2 = ptp.tile([128, 2, 2, 256], BF16, name="pt2", tag="pt")
        nc.scalar.activation(pt2[:], st2[:], AF.Exp)
        for j in range(2):
            pend.append((nkt_sh + j, lambda i, j=j: pt2[:, i, j, :]))
        yield
        while pend:
            flush_pv()
            yield
        # denominators -> reciprocal -> partition broadcast -> normalize
        dstg = tmp.tile([33, 1024], F32, tag="dc")
        nc.gpsimd.memset(dstg[:], 1.0)
        for i in range(2):
            nc.vector.tensor_scalar(out=dstg[0:1, 512 * i:512 * i + 512],
                                    in0=pv[i][64:65, :],
                                    scalar1=1e-30, scalar2=None, op0=ALU.max)
        dr = tmp.tile([33, 1024], F32, tag="dc")
        nc.vector.reciprocal_approx_fast(out=dr[:], in_=dstg[:])
        rd = rdp.tile([64, 1024], F32, tag="rd")
        nc.gpsimd.partition_broadcast(rd[:], dr[0:1, :])
        for i in range(2):
            nc.vector.scalar_tensor_tensor(out=att_sb[hp][64 * i:64 * i + 64, fps],
                                           in0=pv[i][0:64, :], scalar=1.0,
                                           in1=rd[:, 512 * i:512 * i + 512],
                                           op0=MUL, op1=MUL)
        yield

    att_sb = [sing.tile([128, L], BF16, name=f"att{i}") for i in range(2)]

    def finish_fp(fp):
        # casts go to the ACT engine when it has queue slack (copy lives in
        # the same table set as exp, so no table switch)
        on_scalar = False
        for lt4 in range(4):
            lt = fp * 4 + lt4
            lsl = slice(lt * 128, lt * 128 + 128)
            for oc in range(2):
                ocs = slice(oc * 512, oc * 512 + 512)
                ps = pps.tile([128, 512], F32, name="ops", tag="ps")
                for ct in range(2):
                    nc.tensor.matmul(ps[:], att_sb[ct][:, lsl],
                                     wo_sb[:, ct, ocs], start=(ct == 0),
                                     stop=(ct == 1))
                ob = osb.tile([128, 512], BF16, tag="ob")
                if on_scalar:
                    nc.scalar.copy(ob[:], ps[:])
                else:
                    nc.vector.tensor_copy(ob[:], ps[:])
                nc.sync.dma_start(out=out[lt, oc], in_=ob[:])
            yield

    # ---------------- main loop over l-chunks ------------------------------
    def prefetch(nlc):
        xn = xp.tile([128, NDT, 512], BF16, name=f"x{nlc}", tag="xt")
        for dt2 in range(0, NDT, 2):
            nc.sync.dma_start(out=xn[:, dt2:dt2 + 2, :],
                              in_=xTr[nlc - 1, :, dt2:dt2 + 2, :])
        return xn

    xtiles = {0: x0}
    for lc in range(NLC):
        ls = slice(lc * 512, (lc + 1) * 512)
        xt = xtiles.pop(lc)

        if lc == 0:
            for q4 in range(4):
                qs = slice(512 * q4, 512 * q4 + 512)
                nc.scalar.dma_start(out=cos_sb[:, qs], in_=costab[:, qs])
                nc.scalar.dma_start(out=sin_sb[:, qs], in_=sintab[:, qs])
            nc.scalar.dma_start(out=wo_sb[:, 0, :], in_=wo[:, 0, :])
            nc.scalar.dma_start(out=wo_sb[:, 1, :], in_=wo[:, 1, :])
            nc.sync.dma_start(out=wv_sb[:, 0:4, :], in_=wv[:, 0:4, :])
            nc.sync.dma_start(out=wv_sb[:, 4:8, :], in_=wv[:, 4:8, :])

        sqs = []
        for pair in range(2):                    # 0: q (ot 0,1), 1: k (ot 2,3)
            for comp in range(2):
                drv.step(3)
                ot = pair * 2 + comp
                ps = pps.tile([128, 512], F32, name="qkps", tag="ps")
                for dt in range(NDT):
                    nc.tensor.matmul(ps[:], wqk_sb[:, dt, ot * 128:(ot + 1) * 128],
                                     xt[:, dt, :], start=(dt == 0),
                                     stop=(dt == NDT - 1))
                nc.vector.tensor_copy(qk_sb[ot][:, ls], ps[:])
                sq = sqp.tile([128, 512], BF16, name="sq", tag="sq")
                nc.vector.tensor_tensor(sq[:], qk_sb[ot][:, ls],
                                        qk_sb[ot][:, ls], MUL)
                sqs.append(sq)
        if lc + 1 < NLC and lc + 1 not in xtiles:
            xtiles[lc + 1] = prefetch(lc + 1)
        drv.step(2)

        # rms sums: one [8,512] bank; q heads rows 0:4, k heads rows 4:8
        rsum = pps.tile([8, 512], F32, name="rsum", tag="ps")
        for c in range(4):
            nc.tensor.matmul(rsum[:, :], wvec_sb[:, 8 * c:8 * c + 8],
                             sqs[c][:], start=(c == 0), stop=(c == 3),
                             skip_group_check=True)
        # the rms -> round-trip -> rope -> fold chain gates the next fp's exp
        # stream; high_priority stops the list scheduler from parking it
        # behind bulk casts (observed +15us on the first fold)
        with tc.high_priority():
            rln = tmp.tile([8, 512], F32, tag="rln")
            nc.scalar.activation(rln[:], rsum[:], AF.Ln, bias=epsb[:])
            rqs = tmp.tile([8, 512], BF16, tag="rqs")
            nc.scalar.activation(rqs[:], rln[:], AF.Exp, scale=-0.5)
            nc.sync.dma_start(out=skbq[0:8, ls], in_=rqs[0:8, :])
            Rq_b = sing.tile([128, 512], BF16, name=f"rqb{lc}")
            for h in range(4):
                nc.sync.dma_start(out=Rq_b[32 * h:32 * h + 32, :],
                                  in_=skbq[h:h + 1, ls].to_broadcast((32, 512)))
            Rk_b = []
            for hp in range(2):
                rkb = sing.tile([128, 512], BF16, name=f"rkb{lc}_{hp}")
                for i in range(2):
                    nc.sync.dma_start(
                        out=rkb[64 * i:64 * i + 64, :],
                        in_=skbq[4 + 2 * hp + i:5 + 2 * hp + i, ls]
                        .to_broadcast((64, 512)))
                Rk_b.append(rkb)

        # per-lc RoPE, all bf16. K side first and folded immediately (it has
        # no rms-round-trip dependency beyond Rk_b), then V-proj fills the
        # time the Rq_b broadcast needs to land, then the Q side; att2(lc) is
        # born with every dependency already emitted (no caps needed).
        def rope_side(base):
            xr, xi = qk_sb[base][:, ls], qk_sb[base + 1][:, ls]
            for comp in range(2):
                drv.step(2)
                with tc.high_priority():
                    t1 = tmp.tile([128, 512], BF16, tag="t1")
                    t2 = tmp.tile([128, 512], BF16, tag="t2")
                    ca, cb = (cos_sb, sin_sb) if comp == 0 else (sin_sb, cos_sb)
                    nc.vector.tensor_tensor(t1[:], xr, ca[:, ls], MUL)
                    nc.vector.tensor_tensor(t2[:], xi, cb[:, ls], MUL)
                    op = SUB if comp == 0 else ADD
                    dst = rope_sb[base + comp][:, ls]
                    if base == 0:
                        t3 = tmp.tile([128, 512], BF16, tag="t3")
                        nc.vector.tensor_tensor(t3[:], t1[:], t2[:], op)
                        nc.vector.tensor_tensor(dst, t3[:], Rq_b[:], MUL)
                    else:
                        nc.vector.tensor_tensor(dst, t1[:], t2[:], op)
            srcs = (rope_sb[0], rope_sb[1]) if base == 0 else \
                   (rope_sb[2], rope_sb[3])
            dstt = QPl if base == 0 else KPl
            with tc.high_priority():
                for hp2 in range(2):
                    for i2 in range(2):
                        h2 = hp2 * 2 + i2
                        nc.sync.dma_start(
                            out=dstt[hp2][lc][64 * i2:64 * i2 + 32, :],
                            in_=srcs[0][32 * h2:32 * h2 + 32, ls])
                        nc.sync.dma_start(
                            out=dstt[hp2][lc][64 * i2 + 32:64 * i2 + 64, :],
                            in_=srcs[1][32 * h2:32 * h2 + 32, ls])

        # V projection: l on partitions; 2 chains share one bank
        for vt in range(2):
            drv.step(3)
            vps = pps.tile([128, 2, CPG], F32, name="vps", tag="ps")
            for c2 in range(2):
                ls4 = vt * 2 + c2
                for dt in range(NDT):
                    nc.tensor.matmul(vps[:, c2, :],
                                     xt[:, dt, ls4 * 128:(ls4 + 1) * 128],
                                     wv_sb[:, dt, :],
                                     start=(c2 == 0 and dt == 0),
                                     stop=(c2 == 1 and dt == NDT - 1),
                                     skip_group_check=True)
            for c2 in range(2):
                lt = lc * 4 + vt * 2 + c2
                nc.vector.tensor_copy(v_sb[:, lt, :, 0:64], vps[:, c2, :])
        if lc + 2 < NLC:
            xtiles[lc + 2] = prefetch(lc + 2)

        rope_side(2)
        rope_side(0)
        # fp=lc attention can start on old-key tiles now (capped until fold)
        drv.add(att2(0, lc), cap=4 * lc)
        drv.add(att2(1, lc), cap=4 * lc)
        drv.step(2)
        # fold 0.125*r_k into K (per head rows), in place
        with tc.high_priority():
            for hp2 in range(2):
                nc.vector.scalar_tensor_tensor(
                    out=KPl[hp2][lc][:], in0=KPl[hp2][lc][:], scalar=0.125,
                    in1=Rk_b[hp2][:], op0=MUL, op1=MUL)
        drv.uncap_all()
        # fin(fp=lc-1) a few steps into fp=lc's stream so its first matmuls
        # never head-block the tensor queue on the normalize chain
        if lc >= 1:
            drv.add_delayed(finish_fp(lc - 1), after=6)
        drv.step(4)

    # tail: drain everything, then the last out-projection
    drv.add_delayed(finish_fp(3), after=6)
    drv.drain()


def _build_nc():
    import contextlib
    from concourse import hw_specs as _hw

    # Route every Exp/Ln activation to the one table set that holds both
    # (natural_log_exp_and_others), so the per-lc rms Ln doesn't thrash the
    # ACT tables against the attention Exp. Only set *contents* are filtered;
    # list order/length is preserved so act_func_set_id indices stay valid.
    _orig_gat = _hw.get_activation_tables

    def _gat(arch):
        t = _orig_gat(arch)
        both = mybir.ActivationFunctionType.Exp, mybir.ActivationFunctionType.Ln
        return {name: (fns if name == "natural_log_exp_and_others"
                       else fns - set(both))
                for name, fns in t.items()}

    bacc.get_activation_tables = _gat
    nc = bacc.Bacc("TRN2", target_bir_lowering=False, debug=False, num_devices=8)
    xT0 = nc.dram_tensor("xT0", (NDT, 128, 512), BF16, kind="ExternalInput")
    xTr = nc.dram_tensor("xTr", (NLC - 1, 128, NDT, 512), BF16,
                         kind="ExternalInput")
    wqk = nc.dram_tensor("wqk", (NDT, 128, 512), BF16, kind="ExternalInput")
    wv = nc.dram_tensor("wv", (128, NDT, CPG), BF16, kind="ExternalInput")
    wo = nc.dram_tensor("wo", (128, 2, D), BF16, kind="ExternalInput")
    wvec = nc.dram_tensor("wvec", (128, 32), BF16, kind="ExternalInput")
    costab = nc.dram_tensor("costab", (128, L), BF16, kind="ExternalInput")
    sintab = nc.dram_tensor("sintab", (128, L), BF16, kind="ExternalInput")
    out = nc.dram_tensor("out", (NLT, 2, 128, 512), BF16, kind="ExternalOutput")
    skbq = nc.dram_tensor("skbq", (8, L), BF16)

    with tile.TileContext(nc) as tc, contextlib.ExitStack() as ctx:
        _emit(nc, tc, ctx, xT0.ap(), xTr.ap(), wqk.ap(), wv.ap(), wo.ap(),
              wvec.ap(), costab.ap(), sintab.ap(), out.ap(), skbq.ap())
    nc.compile()
    bacc.get_activation_tables = _orig_gat
    return nc


def _host_prep(x, Wqkv, Wout, q_scale, k_scale):
    x = np.asarray(x, np.float32)
    Wqkv = np.asarray(Wqkv, np.float32)
    Wout = np.asarray(Wout, np.float32)
    q_scale = np.asarray(q_scale, np.float32)
    k_scale = np.asarray(k_scale, np.float32)

    quarter = HD // 4  # 16
    inv = 1.0 / (10000.0 ** (np.arange(quarter, dtype=np.float64) / quarter))
    tt = np.repeat(np.arange(T), NP).astype(np.float64)
    pp = np.tile(np.arange(NP), T).astype(np.float64)
    ang = np.concatenate([tt[:, None] * inv[None, :], pp[:, None] * inv[None, :]],
                         axis=1)  # (L, 32)
    costab = np.tile(np.cos(ang).astype(np.float32).T, (4, 1))  # (128, L)
    sintab = np.tile(np.sin(ang).astype(np.float32).T, (4, 1))

    import ml_dtypes
    ev, od = np.arange(0, HD, 2), np.arange(1, HD, 2)
    # four [128,8] rms stationaries (qR,qI,kR,kI); q heads cols 0:4 of each
    # block, k heads cols 4:8; zero-padded so all mms share out rows 0:8
    wvec = np.zeros((128, 32), np.float32)
    for hh in range(HPG):
        r = slice(32 * hh, 32 * hh + 32)
        wvec[r, 0 + hh] = 1.0 / (HD * q_scale[ev] ** 2)
        wvec[r, 8 + hh] = 1.0 / (HD * q_scale[od] ** 2)
        wvec[r, 16 + 4 + hh] = 1.0 / (HD * k_scale[ev] ** 2)
        wvec[r, 24 + 4 + hh] = 1.0 / (HD * k_scale[od] ** 2)

    in_maps = []
    for c in range(8):
        b, g = c // 4, c % 4
        wqk = np.empty((D, 512), np.float32)
        for hh in range(HPG):
            gh = g * HPG + hh
            wq = Wqkv[gh * HD:(gh + 1) * HD, :] * q_scale[:, None]
            wk = Wqkv[D + gh * HD:D + (gh + 1) * HD, :] * k_scale[:, None]
            wqk[:, 0 + 32 * hh:32 + 32 * hh] = wq[ev].T
            wqk[:, 128 + 32 * hh:160 + 32 * hh] = wq[od].T
            wqk[:, 256 + 32 * hh:288 + 32 * hh] = wk[ev].T
            wqk[:, 384 + 32 * hh:416 + 32 * hh] = wk[od].T
        # pre-tiled layouts: contiguous per-tile DRAM so the DMA descriptor
        # stream stays compact
        wqk_t = np.ascontiguousarray(wqk.reshape(NDT, 128, 512))
        wv_m = Wqkv[2 * D + g * CPG:2 * D + (g + 1) * CPG, :].T  # (D, CPG)
        wv_t = np.ascontiguousarray(wv_m.reshape(NDT, 128, CPG).transpose(1, 0, 2))
        wo_m = Wout[:, g * CPG:(g + 1) * CPG].T                  # (CPG, D)
        wo_t = np.ascontiguousarray(wo_m.reshape(2, 128, D).transpose(1, 0, 2))
        xT = x[b].T                                              # (D, L)
        # lc0 dt-major contiguous tiles; lc1-3 p-major (one dma per chunk)
        x0_t = np.ascontiguousarray(
            xT[:, 0:512].reshape(NDT, 128, 512))
        xr_t = np.ascontiguousarray(
            xT[:, 512:].reshape(NDT, 128, NLC - 1, 512).transpose(2, 1, 0, 3))
        in_maps.append({
            "xT0": x0_t.astype(ml_dtypes.bfloat16),
            "xTr": xr_t.astype(ml_dtypes.bfloat16),
            "wqk": wqk_t.astype(ml_dtypes.bfloat16),
            "wv": wv_t.astype(ml_dtypes.bfloat16),
            "wo": wo_t.astype(ml_dtypes.bfloat16),
            "wvec": wvec.astype(ml_dtypes.bfloat16),
            "costab": costab.astype(ml_dtypes.bfloat16),
            "sintab": sintab.astype(ml_dtypes.bfloat16),
        })
    return in_maps


def kernel(x, Wqkv, Wout, q_scale, k_scale, T=None, N_p=None):
    assert int(T) == 8 and int(N_p) == 256
    if "nc" not in _CACHE:
        _CACHE["nc"] = _build_nc()
    nc = _CACHE["nc"]
    in_maps = _host_prep(x, Wqkv, Wout, q_scale, k_scale)
    trace = bool(int(os.environ.get("KERNEL_TRACE", "0")))
    res = run_bass_kernel_spmd(nc, in_maps, core_ids=list(range(8)), trace=trace)
    _CACHE["last_exec_time_ns"] = res.exec_time_ns
    outp = np.zeros((B, L, D), np.float32)
    for c in range(8):
        o = np.asarray(res.results[c]["out"], np.float32)  # (NLT, 2, 128, 512)
        outp[c // 4] += o.transpose(0, 2, 1, 3).reshape(L, D)
    return outp


if __name__ == "__main__":
    rng = np.random.default_rng(0)
    x = rng.standard_normal((B, L, D), dtype=np.float32)
    Wqkv = rng.standard_normal((3 * D, D), dtype=np.float32) * 0.02
    Wout = rng.standard_normal((D, D), dtype=np.float32) * 0.02
    o = kernel(x, Wqkv, Wout, np.ones(HD, np.float32), np.ones(HD, np.float32),
               8, 256)
    print("out", o.shape, o.dtype, float(np.abs(o).mean()))


# revision 72
# speedup vs baseline: 1.2001x; 1.0016x over previous
"""Block-causal attention Trainium2 kernel (8 NeuronCores), v3.

Sharding: core c = b*4 + g handles batch b (of 2) and head-group g (4 of 16
heads). Each core computes the qkv projection, rmsnorm + 2-D RoPE,
block-causal attention and a partial output projection for its 256 channels;
the host sums the 4 per-group partials per batch (bf16 partials).

v3 (222.6us -> ~196us). NOTE: the walrus schedule is sensitive to source
  LINE NUMBERS -- edits below reshuffle it +-5us; keep line counts fixed.
  merged exps: per-kt head-pair scores go into one [128,2,512] psum tile;
    ONE exp covers both heads (72x1067ns=77us, the serial wall; tail-pair
    kts use the same shape).
  normalize via ONE gpsimd partition_broadcast (no skbd DRAM round-trip).
  psum: stp 2x2-bank + pvp 2 + pps 2 = 8 banks; head-pairs driven
    sequentially (hp0 fully, then hp1) so 2 pv banks suffice.
  pull-forward: att(fp) starts during lc=fp (old keys, capped at 4*fp kts
    until KPl[lc] folds); fin(fp) enters the driver a few steps into fp+1.
  pre-tiled DRAM (contiguous [128,512] tiles for x/wqk/wv/wo/out) kills the
    startup descriptor storm; 60 warmup matmuls (stationary from a gpsimd
    memset -> start at engine-go) keep the PE HAM-warm through DMA wait.
  rms skbq round-trip on sync DMA rings (parallel ~2us, not ~5us/lc on
    Pool); x prefetched 2 deep in split halves; V-copies fused [128,4x64].
A/B facts: fin interleaved INTO att2 (via pps ring) head-blocks the tensor
  queue; K-fold must trail V-proj or its Rk_b wait head-blocks the vector
  queue; <60 warmups lets the HAM re-throttle early (+8us).
Hardware pitfalls baked in: tile_position column-tiling produces garbage on
  this HW; multi-bank PSUM *matmul out* tiles are fine per-bank-slice and a
  single ACT may READ across 2 banks (verified); partition_broadcast only
  sources partition 0; reciprocal_approx_fast needs multi-row base-0 fp32
  APs; GPSIMD (Pool) cannot touch PSUM, only copy/memset-class ops.
"""

import os
import numpy as np

import concourse.bass as bass
import concourse.mybir as mybir
import concourse.tile as tile
from concourse import bacc
from concourse.bass_utils import run_bass_kernel_spmd

F32 = mybir.dt.float32
BF16 = mybir.dt.bfloat16
AF = mybir.ActivationFunctionType
ALU = mybir.AluOpType
MUL = ALU.mult
ADD = ALU.add
SUB = ALU.subtract

B, T, NP, D, H = 2, 8, 256, 1024, 16
L = T * NP            # 2048
HD = 64               # head dim
HPG = 4               # heads per group (4 groups x 2 batches = 8 cores)
CPG = HPG * HD        # 256 channels per group
NDT = D // 128        # 8 d-tiles
NLC = L // 512        # 4 l-chunks (= frame pairs)
NLT = L // 128        # 16 l-tiles
EPS = 1e-6

_CACHE = {}


class Driver:
    """FIFO generator driver with per-entry caps and delayed insertion."""

    def __init__(self):
        self.q = []        # [gen, cap_or_None]
        self.delayed = []  # [steps_left, gen]

    def add(self, gen, cap=None):
        self.q.append([gen, cap])

    def add_delayed(self, gen, after=4):
        self.delayed.append([after, gen])

    def uncap_all(self):
        for e in self.q:
            e[1] = None

    def step(self, n=1):
        done = 0
        while done < n:
            ent = None
            for e in self.q:
                if e[1] is None or e[1] > 0:
                    ent = e
                    break
            if ent is None:
                if self.delayed:
                    d = self.delayed.pop(0)
                    self.q.append([d[1], None])
                    continue
                return done
            r = next(ent[0], "END")
            if r == "END":
                self.q.remove(ent)
            elif ent[1] is not None:
                ent[1] -= 1
            done += 1
            for d in self.delayed:
                d[0] -= 1
            while self.delayed and self.delayed[0][0] <= 0:
                d = self.delayed.pop(0)
                self.q.append([d[1], None])
        return done

    def drain(self):
        while self.step(32):
            pass


def _emit(nc, tc, ctx, xT0, xTr, wqk, wv, wo, wvec, costab, sintab, out, skbq):
    sing = ctx.enter_context(tc.tile_pool(name="sing", bufs=1))
    xp = ctx.enter_context(tc.tile_pool(name="xp", bufs=3))
    tmp = ctx.enter_context(tc.tile_pool(name="tmp", bufs=4))
    sqp = ctx.enter_context(tc.tile_pool(name="sqp", bufs=4))
    ptp = ctx.enter_context(tc.tile_pool(name="ptp", bufs=5))
    osb = ctx.enter_context(tc.tile_pool(name="osb", bufs=4))
    rdp = ctx.enter_context(tc.tile_pool(name="rdp", bufs=1))
    # PSUM: pps 2 + stp 2x2 + pvp 2 = 8 banks
    pps = ctx.enter_context(tc.tile_pool(name="pps", bufs=2, space="PSUM"))
    stp = ctx.enter_context(tc.tile_pool(name="stp", bufs=2, space="PSUM"))
    pvp = ctx.enter_context(tc.tile_pool(name="pvp", bufs=2, space="PSUM"))

    # ---- persistent SBUF; first-needed first ----
    wvec_sb = sing.tile([128, 32], BF16)
    nc.sync.dma_start(out=wvec_sb[:], in_=wvec[:])
    # warmup matmuls: keep the PE HAM-warm through the input DMA wait; the
    # stationary comes from a gpsimd memset (that queue is empty at start)
    wrm = sing.tile([128, 128], BF16)
    nc.gpsimd.memset(wrm[:], 0.125)
    wps = pps.tile([128, 128], F32, name="wps", tag="ps")
    for i in range(60):
        nc.tensor.matmul(wps[:], wrm[:], wrm[:], start=True, stop=True,
                         skip_group_check=True)
    # x0/wqk split into 64KB pieces so they spread over all DMA rings and
    # land first; everything else (x1, wv, cos/sin/wo) is emitted later from
    # the lc0 section body so it can't steal their bandwidth
    wqk_sb = sing.tile([128, NDT, 512], BF16)
    x0 = xp.tile([128, NDT, 512], BF16, name="x0", tag="xt")
    for dt in range(NDT):
        for h in range(2):
            cs = slice(256 * h, 256 * h + 256)
            nc.sync.dma_start(out=x0[:, dt, cs], in_=xT0[dt, :, cs])
            nc.sync.dma_start(out=wqk_sb[:, dt, cs], in_=wqk[dt, :, cs])
    wv_sb = sing.tile([128, NDT, CPG], BF16)
    # cos/sin/wo are loaded from inside the lc0 section (deferred so they
    # don't compete with the startup-critical x0/wqk loads)
    cos_sb = sing.tile([128, L], BF16)
    sin_sb = sing.tile([128, L], BF16)
    wo_sb = sing.tile([128, 2, D], BF16)

    qk_sb = [sing.tile([128, L], BF16, name=f"qk{i}") for i in range(4)]
    rope_sb = [sing.tile([128, L], BF16, name=f"rope{i}") for i in range(4)]
    QPl = [[sing.tile([128, 512], BF16, name=f"qp{i}_{c}") for c in range(NLC)]
           for i in range(2)]
    KPl = [[sing.tile([128, 512], BF16, name=f"kp{i}_{c}") for c in range(NLC)]
           for i in range(2)]
    v_sb = sing.tile([128, NLT, 4, 65], BF16)
    nc.vector.memset(v_sb[:, :, :, 64:65], 1.0)

    epsb = sing.tile([8, 1], F32)
    nc.vector.memset(epsb[:], EPS)

    drv = Driver()

    # ---------------- attention pass (generator, one head-pair) ------------
    def att2(hp, fp):
        nkt_sh, nkt_all = 4 * fp + 2, 4 * fp + 4
        last = nkt_all - 1
        fps = slice(512 * fp, 512 * fp + 512)
        pv = [pvp.tile([65, 512], F32, name=f"pv{hp}_{fp}_{i}", tag="pv")
              for i in range(2)]
        pend = []

        def flush_pv():
            kt_, rhs = pend.pop(0)
            qof_ = 0 if kt_ < nkt_sh else 256
            for i in range(2):
                nc.tensor.matmul(pv[i][:, qof_:512],
                                 v_sb[:, kt_, 2 * hp + i, :], rhs(i),
                                 start=(kt_ == 0), stop=(kt_ == last),
                                 skip_group_check=True)

        for kt in range(nkt_sh):
            lck, kof = kt // 4, (kt % 4) * 128
            st = stp.tile([128, 2, 512], F32, name="st", tag="st")
            for i in range(2):
                nc.tensor.matmul(st[:, i, :],
                                 KPl[hp][lck][64 * i:64 * i + 64, kof:kof + 128],
                                 QPl[hp][fp][64 * i:64 * i + 64, :],
                                 start=True, stop=True, skip_group_check=True)
            pt = ptp.tile([128, 2, 512], BF16, name="pt", tag="pt")
            nc.scalar.activation(pt[:], st[:], AF.Exp)
            pend.append((kt, lambda i, p=pt: p[:, i, 0:512]))
            if len(pend) > 1:
                flush_pv()
            yield
        # trailing two key-tiles (frame f1 only, queries 256:512): both heads
        # and both kts in one [128,2,2,256] tile -> a single exp
        st2 = stp.tile([128, 2, 2, 256], F32, name="st2", tag="st")
        for j in range(2):
            kt = nkt_sh + j
            lck, kof = kt // 4, (kt % 4) * 128
            for i in range(2):
                nc.tensor.matmul(st2[:, i, j, :],
                                 KPl[hp][lck][64 * i:64 * i + 64, kof:kof + 128],
                                 QPl[hp][fp][64 * i:64 * i + 64, 256:512],
                                 start=True, stop=True, skip_group_check=True)
        pt2 = ptp.tile([128, 2, 2, 256], BF16, name="pt2", tag="pt")
        nc.scalar.activation(pt2[:], st2[:], AF.Exp)
        for j in range(2):
            pend.append((nkt_sh + j, lambda i, j=j: pt2[:, i, j, :]))
        yield
        while pend:
            flush_pv()
            yield
        # denominators -> reciprocal -> partition broadcast -> normalize
        dstg = tmp.tile([33, 1024], F32, tag="dc")
        nc.gpsimd.memset(dstg[:], 1.0)
        for i in range(2):
            nc.vector.tensor_scalar(out=dstg[0:1, 512 * i:512 * i + 512],
                                    in0=pv[i][64:65, :],
                                    scalar1=1e-30, scalar2=None, op0=ALU.max)
        dr = tmp.tile([33, 1024], F32, tag="dc")
        nc.vector.reciprocal_approx_fast(out=dr[:], in_=dstg[:])
        rd = rdp.tile([64, 1024], F32, tag="rd")
        nc.gpsimd.partition_broadcast(rd[:], dr[0:1, :])
        for i in range(2):
            nc.vector.scalar_tensor_tensor(out=att_sb[hp][64 * i:64 * i + 64, fps],
                                           in0=pv[i][0:64, :], scalar=1.0,
                                           in1=rd[:, 512 * i:512 * i + 512],
                                           op0=MUL, op1=MUL)
        yield

    att_sb = [sing.tile([128, L], BF16, name=f"att{i}") for i in range(2)]

    def finish_fp(fp):
        # casts go to the ACT engine when it has queue slack (copy lives in
        # the same table set as exp, so no table switch)
        on_scalar = False
        for lt4 in range(4):
            lt = fp * 4 + lt4
            lsl = slice(lt * 128, lt * 128 + 128)
            for oc in range(2):
                ocs = slice(oc * 512, oc * 512 + 512)
                ps = pps.tile([128, 512], F32, name="ops", tag="ps")
                for ct in range(2):
                    nc.tensor.matmul(ps[:], att_sb[ct][:, lsl],
                                     wo_sb[:, ct, ocs], start=(ct == 0),
                                     stop=(ct == 1))
                ob = osb.tile([128, 512], BF16, tag="ob")
                if on_scalar:
                    nc.scalar.copy(ob[:], ps[:])
                else:
                    nc.vector.tensor_copy(ob[:], ps[:])
                nc.sync.dma_start(out=out[lt, oc], in_=ob[:])
            yield

    # ---------------- main loop over l-chunks ------------------------------
    def prefetch(nlc):
        xn = xp.tile([128, NDT, 512], BF16, name=f"x{nlc}", tag="xt")
        for dt2 in range(0, NDT, 2):
            nc.sync.dma_start(out=xn[:, dt2:dt2 + 2, :],
                              in_=xTr[nlc - 1, :, dt2:dt2 + 2, :])
        return xn

    xtiles = {0: x0}
    for lc in range(NLC):
        ls = slice(lc * 512, (lc + 1) * 512)
        xt = xtiles.pop(lc)

        if lc == 0:
            for q4 in range(4):
                qs = slice(512 * q4, 512 * q4 + 512)
                nc.scalar.dma_start(out=cos_sb[:, qs], in_=costab[:, qs])
                nc.scalar.dma_start(out=sin_sb[:, qs], in_=sintab[:, qs])
            nc.scalar.dma_start(out=wo_sb[:, 0, :], in_=wo[:, 0, :])
            nc.scalar.dma_start(out=wo_sb[:, 1, :], in_=wo[:, 1, :])
            nc.sync.dma_start(out=wv_sb[:, 0:4, :], in_=wv[:, 0:4, :])
            nc.sync.dma_start(out=wv_sb[:, 4:8, :], in_=wv[:, 4:8, :])

        sqs = []
        for pair in range(2):                    # 0: q (ot 0,1), 1: k (ot 2,3)
            for comp in range(2):
                drv.step(3)
                ot = pair * 2 + comp
                ps = pps.tile([128, 512], F32, name="qkps", tag="ps")
                for dt in range(NDT):
                    nc.tensor.matmul(ps[:], wqk_sb[:, dt, ot * 128:(ot + 1) * 128],
                                     xt[:, dt, :], start=(dt == 0),
                                     stop=(dt == NDT - 1))
                nc.vector.tensor_copy(qk_sb[ot][:, ls], ps[:])
                sq = sqp.tile([128, 512], BF16, name="sq", tag="sq")
                nc.vector.tensor_tensor(sq[:], qk_sb[ot][:, ls],
                                        qk_sb[ot][:, ls], MUL)
                sqs.append(sq)
        if lc + 1 < NLC and lc + 1 not in xtiles:
            xtiles[lc + 1] = prefetch(lc + 1)
        drv.step(2)

        # rms sums: one [8,512] bank; q heads rows 0:4, k heads rows 4:8
        rsum = pps.tile([8, 512], F32, name="rsum", tag="ps")
        for c in range(4):
            nc.tensor.matmul(rsum[:, :], wvec_sb[:, 8 * c:8 * c + 8],
                             sqs[c][:], start=(c == 0), stop=(c == 3),
                             skip_group_check=True)
        # the rms -> round-trip -> rope -> fold chain gates the next fp's exp
        # stream; high_priority stops the list scheduler from parking it
        # behind bulk casts (observed +15us on the first fold)
        with tc.high_priority():
            rln = tmp.tile([8, 512], F32, tag="rln")
            nc.scalar.activation(rln[:], rsum[:], AF.Ln, bias=epsb[:])
            rqs = tmp.tile([8, 512], BF16, tag="rqs")
            nc.scalar.activation(rqs[:], rln[:], AF.Exp, scale=-0.5)
            nc.sync.dma_start(out=skbq[0:8, ls], in_=rqs[0:8, :])
            Rq_b = sing.tile([128, 512], BF16, name=f"rqb{lc}")
            for h in range(4):
                nc.sync.dma_start(out=Rq_b[32 * h:32 * h + 32, :],
                                  in_=skbq[h:h + 1, ls].to_broadcast((32, 512)))
            Rk_b = []
            for hp in range(2):
                rkb = sing.tile([128, 512], BF16, name=f"rkb{lc}_{hp}")
                for i in range(2):
                    nc.sync.dma_start(
                        out=rkb[64 * i:64 * i + 64, :],
                        in_=skbq[4 + 2 * hp + i:5 + 2 * hp + i, ls]
                        .to_broadcast((64, 512)))
                Rk_b.append(rkb)

        # per-lc RoPE, all bf16. K side first and folded immediately (it has
        # no rms-round-trip dependency beyond Rk_b), then V-proj fills the
        # time the Rq_b broadcast needs to land, then the Q side; att2(lc) is
        # born with every dependency already emitted (no caps needed).
        def rope_side(base):
            xr, xi = qk_sb[base][:, ls], qk_sb[base + 1][:, ls]
            for comp in range(2):
                drv.step(2)
                with tc.high_priority():
                    t1 = tmp.tile([128, 512], BF16, tag="t1")
                    t2 = tmp.tile([128, 512], BF16, tag="t2")
                    ca, cb = (cos_sb, sin_sb) if comp == 0 else (sin_sb, cos_sb)
                    nc.vector.tensor_tensor(t1[:], xr, ca[:, ls], MUL)
                    nc.vector.tensor_tensor(t2[:], xi, cb[:, ls], MUL)
                    op = SUB if comp == 0 else ADD
                    dst = rope_sb[base + comp][:, ls]
                    if base == 0:
                        t3 = tmp.tile([128, 512], BF16, tag="t3")
                        nc.vector.tensor_tensor(t3[:], t1[:], t2[:], op)
                        nc.vector.tensor_tensor(dst, t3[:], Rq_b[:], MUL)
                    else:
                        nc.vector.tensor_tensor(dst, t1[:], t2[:], op)
            srcs = (rope_sb[0], rope_sb[1]) if base == 0 else \
                   (rope_sb[2], rope_sb[3])
            dstt = QPl if base == 0 else KPl
            with tc.high_priority():
                for hp2 in range(2):
                    for i2 in range(2):
                        h2 = hp2 * 2 + i2
                        nc.sync.dma_start(
                            out=dstt[hp2][lc][64 * i2:64 * i2 + 32, :],
                            in_=srcs[0][32 * h2:32 * h2 + 32, ls])
                        nc.sync.dma_start(
                            out=dstt[hp2][lc][64 * i2 + 32:64 * i2 + 64, :],
                            in_=srcs[1][32 * h2:32 * h2 + 32, ls])

        # V projection: l on partitions; 2 chains share one bank
        for vt in range(2):
            drv.step(3)
            vps = pps.tile([128, 2, CPG], F32, name="vps", tag="ps")
            for c2 in range(2):
                ls4 = vt * 2 + c2
                for dt in range(NDT):
                    nc.tensor.matmul(vps[:, c2, :],
                                     xt[:, dt, ls4 * 128:(ls4 + 1) * 128],
                                     wv_sb[:, dt, :],
                                     start=(c2 == 0 and dt == 0),
                                     stop=(c2 == 1 and dt == NDT - 1),
                                     skip_group_check=True)
            for c2 in range(2):
                lt = lc * 4 + vt * 2 + c2
                nc.vector.tensor_copy(v_sb[:, lt, :, 0:64], vps[:, c2, :])
        if lc + 2 < NLC:
            xtiles[lc + 2] = prefetch(lc + 2)

        rope_side(2)
        rope_side(0)
        # fp=lc attention can start on old-key tiles now (capped until fold)
        drv.add(att2(0, lc), cap=4 * lc)
        drv.add(att2(1, lc), cap=4 * lc)
        drv.step(2)
        # fold 0.125*r_k into K (per head rows), in place
        with tc.high_priority():
            for hp2 in range(2):
                nc.vector.scalar_tensor_tensor(
                    out=KPl[hp2][lc][:], in0=KPl[hp2][lc][:], scalar=0.125,
                    in1=Rk_b[hp2][:], op0=MUL, op1=MUL)
        drv.uncap_all()
        # fin(fp=lc-1) a few steps into fp=lc's stream so its first matmuls
        # never head-block the tensor queue on the normalize chain
        if lc >= 1:
            drv.add_delayed(finish_fp(lc - 1), after=6)
        drv.step(4)

    # tail: drain everything, then the last out-projection
    drv.add_delayed(finish_fp(3), after=6)
    drv.drain()


def _build_nc():
    import contextlib
    from concourse import hw_specs as _hw

    # Route every Exp/Ln activation to the one table set that holds both
    # (natural_log_exp_and_others), so the per-lc rms Ln doesn't thrash the
    # ACT tables against the attention Exp. Only set *contents* are filtered;
    # list order/length is preserved so act_func_set_id indices stay valid.
    _orig_gat = _hw.get_activation_tables

    def _gat(arch):
        t = _orig_gat(arch)
        both = mybir.ActivationFunctionType.Exp, mybir.ActivationFunctionType.Ln
        return {name: (fns if name == "natural_log_exp_and_others"
                       else fns - set(both))
                for name, fns in t.items()}

    bacc.get_activation_tables = _gat
    nc = bacc.Bacc("TRN2", target_bir_lowering=False, debug=False, num_devices=8)
    xT0 = nc.dram_tensor("xT0", (NDT, 128, 512), BF16, kind="ExternalInput")
    xTr = nc.dram_tensor("xTr", (NLC - 1, 128, NDT, 512), BF16,
                         kind="ExternalInput")
    wqk = nc.dram_tensor("wqk", (NDT, 128, 512), BF16, kind="ExternalInput")
    wv = nc.dram_tensor("wv", (128, NDT, CPG), BF16, kind="ExternalInput")
    wo = nc.dram_tensor("wo", (128, 2, D), BF16, kind="ExternalInput")
    wvec = nc.dram_tensor("wvec", (128, 32), BF16, kind="ExternalInput")
    costab = nc.dram_tensor("costab", (128, L), BF16, kind="ExternalInput")
    sintab = nc.dram_tensor("sintab", (128, L), BF16, kind="ExternalInput")
    out = nc.dram_tensor("out", (NLT, 2, 128, 512), BF16, kind="ExternalOutput")
    skbq = nc.dram_tensor("skbq", (8, L), BF16)

    with tile.TileContext(nc) as tc, contextlib.ExitStack() as ctx:
        _emit(nc, tc, ctx, xT0.ap(), xTr.ap(), wqk.ap(), wv.ap(), wo.ap(),
              wvec.ap(), costab.ap(), sintab.ap(), out.ap(), skbq.ap())
    nc.compile()
    bacc.get_activation_tables = _orig_gat
    return nc


def _host_prep(x, Wqkv, Wout, q_scale, k_scale):
    x = np.asarray(x, np.float32)
    Wqkv = np.asarray(Wqkv, np.float32)
    Wout = np.asarray(Wout, np.float32)
    q_scale = np.asarray(q_scale, np.float32)
    k_scale = np.asarray(k_scale, np.float32)

    quarter = HD // 4  # 16
    inv = 1.0 / (10000.0 ** (np.arange(quarter, dtype=np.float64) / quarter))
    tt = np.repeat(np.arange(T), NP).astype(np.float64)
    pp = np.tile(np.arange(NP), T).astype(np.float64)
    ang = np.concatenate([tt[:, None] * inv[None, :], pp[:, None] * inv[None, :]],
                         axis=1)  # (L, 32)
    costab = np.tile(np.cos(ang).astype(np.float32).T, (4, 1))  # (128, L)
    sintab = np.tile(np.sin(ang).astype(np.float32).T, (4, 1))

    import ml_dtypes
    ev, od = np.arange(0, HD, 2), np.arange(1, HD, 2)
    # four [128,8] rms stationaries (qR,qI,kR,kI); q heads cols 0:4 of each
    # block, k heads cols 4:8; zero-padded so all mms share out rows 0:8
    wvec = np.zeros((128, 32), np.float32)
    for hh in range(HPG):
        r = slice(32 * hh, 32 * hh + 32)
        wvec[r, 0 + hh] = 1.0 / (HD * q_scale[ev] ** 2)
        wvec[r, 8 + hh] = 1.0 / (HD * q_scale[od] ** 2)
        wvec[r, 16 + 4 + hh] = 1.0 / (HD * k_scale[ev] ** 2)
        wvec[r, 24 + 4 + hh] = 1.0 / (HD * k_scale[od] ** 2)

    in_maps = []
    for c in range(8):
        b, g = c // 4, c % 4
        wqk = np.empty((D, 512), np.float32)
        for hh in range(HPG):
            gh = g * HPG + hh
            wq = Wqkv[gh * HD:(gh + 1) * HD, :] * q_scale[:, None]
            wk = Wqkv[D + gh * HD:D + (gh + 1) * HD, :] * k_scale[:, None]
            wqk[:, 0 + 32 * hh:32 + 32 * hh] = wq[ev].T
            wqk[:, 128 + 32 * hh:160 + 32 * hh] = wq[od].T
            wqk[:, 256 + 32 * hh:288 + 32 * hh] = wk[ev].T
            wqk[:, 384 + 32 * hh:416 + 32 * hh] = wk[od].T
        # pre-tiled layouts: contiguous per-tile DRAM so the DMA descriptor
        # stream stays compact
        wqk_t = np.ascontiguousarray(wqk.reshape(NDT, 128, 512))
        wv_m = Wqkv[2 * D + g * CPG:2 * D + (g + 1) * CPG, :].T  # (D, CPG)
        wv_t = np.ascontiguousarray(wv_m.reshape(NDT, 128, CPG).transpose(1, 0, 2))
        wo_m = Wout[:, g * CPG:(g + 1) * CPG].T                  # (CPG, D)
        wo_t = np.ascontiguousarray(wo_m.reshape(2, 128, D).transpose(1, 0, 2))
        xT = x[b].T                                              # (D, L)
        # lc0 dt-major contiguous tiles; lc1-3 p-major (one dma per chunk)
        x0_t = np.ascontiguousarray(
            xT[:, 0:512].reshape(NDT, 128, 512))
        xr_t = np.ascontiguousarray(
            xT[:, 512:].reshape(NDT, 128, NLC - 1, 512).transpose(2, 1, 0, 3))
        in_maps.append({
            "xT0": x0_t.astype(ml_dtypes.bfloat16),
            "xTr": xr_t.astype(ml_dtypes.bfloat16),
            "wqk": wqk_t.astype(ml_dtypes.bfloat16),
            "wv": wv_t.astype(ml_dtypes.bfloat16),
            "wo": wo_t.astype(ml_dtypes.bfloat16),
            "wvec": wvec.astype(ml_dtypes.bfloat16),
            "costab": costab.astype(ml_dtypes.bfloat16),
            "sintab": sintab.astype(ml_dtypes.bfloat16),
        })
    return in_maps


def kernel(x, Wqkv, Wout, q_scale, k_scale, T=None, N_p=None):
    assert int(T) == 8 and int(N_p) == 256
    if "nc" not in _CACHE:
        _CACHE["nc"] = _build_nc()
    nc = _CACHE["nc"]
    in_maps = _host_prep(x, Wqkv, Wout, q_scale, k_scale)
    trace = bool(int(os.environ.get("KERNEL_TRACE", "0")))
    res = run_bass_kernel_spmd(nc, in_maps, core_ids=list(range(8)), trace=trace)
    _CACHE["last_exec_time_ns"] = res.exec_time_ns
    outp = np.zeros((B, L, D), np.float32)
    for c in range(8):
        o = np.asarray(res.results[c]["out"], np.float32)  # (NLT, 2, 128, 512)
        outp[c // 4] += o.transpose(0, 2, 1, 3).reshape(L, D)
    return outp


if __name__ == "__main__":
    rng = np.random.default_rng(0)
    x = rng.standard_normal((B, L, D), dtype=np.float32)
    Wqkv = rng.standard_normal((3 * D, D), dtype=np.float32) * 0.02
    Wout = rng.standard_normal((D, D), dtype=np.float32) * 0.02
    o = kernel(x, Wqkv, Wout, np.ones(HD, np.float32), np.ones(HD, np.float32),
               8, 256)
    print("out", o.shape, o.dtype, float(np.abs(o).mean()))
